# revision 8
# baseline (speedup 1.0000x reference)
"""ConditionalWaveNet Trainium2 kernel.

Sharding: data-parallel over batch B=8 across 8 NeuronCores (one batch
element per core). Within a core the T=16384 sequence is split into two
halves of 8192 laid out on SBUF partition rows 0:64 / 64:128 so all 128
partitions (and a full 128x128 stationary via block-diagonal weights) are
used despite C=64 channels. Dilated causal convs (k=2, d in {1,2,4,8})
read column-shifted slices of a left-padded h buffer; the half-boundary
needs the tail of half A, which a tiny corrective matmul accumulates into
the first d columns of half B's output. Conditioning (embedding lookup +
per-block linear) is a per-(batch, block) channel bias; it is folded on
the host into the conv bias vectors.

Numerics: matmuls run in float16 (full PE rate, 10-bit mantissa; fp32
PSUM accumulate). The residual stream h is kept in fp32; a ping-pong
float16 mirror of h feeds the convs (ping-pong also gives the convs a
stable snapshot of the previous block's h across tile boundaries).
"""

import numpy as np

B, T, NCH, C = 8, 16384, 256, 64
H = T // 2
PAD = 8
TILE = 512
NT = H // TILE  # 16
DILS = (1, 2, 4, 8) * 3
NBLK = len(DILS)
NW = 93  # packed 128x128 stationaries
NBV = 40  # packed bias vectors
NCORES = 8

MM_MODE = "f16"  # "f16" | "bf16" | "f32"
_NP_CDT = {"f16": np.float16, "bf16": None, "f32": np.float32}

_built = None


# ---------------------------------------------------------------- host packing

def _np_cdt():
    if MM_MODE == "bf16":
        import ml_dtypes
        return ml_dtypes.bfloat16
    return _NP_CDT[MM_MODE]


def _bd(w):
    """[64,64] W (out,in) -> block-diag lhsT [128,128] = diag(W.T, W.T)."""
    m = np.zeros((128, 128), np.float32)
    m[:C, :C] = w.T
    m[C:, C:] = w.T
    return m


def _corr(w0):
    """Corrective stationary: rows 0:64 (K from half-A tail) -> cols 64:128
    (output partitions of half B)."""
    m = np.zeros((128, 128), np.float32)
    m[:C, C:] = w0.T
    return m


def _pack_weights(in_w, fw, gw, ow, o1w, o2w):
    ws = np.zeros((NW, 128, 128), np.float32)
    for c in range(4):
        ws[c] = _bd(in_w[:, C * c:C * (c + 1)])
    for i in range(NBLK):
        b = 4 + 7 * i
        ws[b + 0] = _bd(fw[i, :, :, 0])
        ws[b + 1] = _bd(fw[i, :, :, 1])
        ws[b + 2] = _bd(gw[i, :, :, 0])
        ws[b + 3] = _bd(gw[i, :, :, 1])
        ws[b + 4] = _corr(fw[i, :, :, 0])
        ws[b + 5] = _corr(gw[i, :, :, 0])
        ws[b + 6] = _bd(ow[i])
    ws[88] = _bd(o1w)
    o2wT = o2w.T  # [64, 256]
    for oc in range(2):
        for hf in range(2):
            m = np.zeros((128, 128), np.float32)
            m[C * hf:C * (hf + 1), :] = o2wT[:, 128 * oc:128 * (oc + 1)]
            ws[89 + 2 * oc + hf] = m
    # [NW,128k,128m] -> SBUF layout [128 part(k), NW*128]
    out = ws.transpose(1, 0, 2).reshape(128, NW * 128)
    return np.ascontiguousarray(out.astype(_np_cdt()))


def _pack_bvec(cond_b, in_b, fb, cfw, cfb, gb, cgw, cgb, ob, o1b, o2b):
    bv = np.zeros((NBV, 128), np.float32)

    def two(v):
        return np.concatenate([v, v])

    bv[0] = two(in_b)
    for i in range(NBLK):
        bv[1 + i] = two(fb[i] + cfw[i] @ cond_b + cfb[i])
        bv[13 + i] = two(gb[i] + cgw[i] @ cond_b + cgb[i])
        bv[25 + i] = two(ob[i])
    bv[37] = two(o1b)
    bv[38] = o2b[:128]
    bv[39] = o2b[128:]
    return np.ascontiguousarray(bv.T)  # [128, NBV]


def _pack_x(xb):
    """x[b] [256, 16384] -> [128, 4*H]; row p = half(p//64)*64+ch(p%64),
    col = chunk*H + t."""
    a = xb.reshape(4, C, 2, H)        # [chunk, ch, half, t]
    out = a.transpose(2, 1, 0, 3).reshape(128, 4 * H)
    return np.ascontiguousarray(out.astype(_np_cdt()))


# ---------------------------------------------------------------- bass build

def _build():
    global _built
    if _built is not None:
        return _built
    from contextlib import ExitStack

    import concourse.mybir as mybir
    import concourse.tile as tile
    from concourse import bacc

    dt = mybir.dt
    f32 = dt.float32
    cdt = {"f16": dt.float16, "bf16": dt.bfloat16, "f32": dt.float32}[MM_MODE]
    AF = mybir.ActivationFunctionType
    ADD = mybir.AluOpType.add

    nc = bacc.Bacc()
    xs = nc.dram_tensor("xs", [128, 4 * H], cdt, kind="ExternalInput")
    wm = nc.dram_tensor("wm", [128, NW * 128], cdt, kind="ExternalInput")
    bvd = nc.dram_tensor("bv", [128, NBV], f32, kind="ExternalInput")
    y = nc.dram_tensor("y", [2, 128, T], f32, kind="ExternalOutput")

    with ExitStack() as es:
        tc = es.enter_context(tile.TileContext(nc))
        singles = es.enter_context(tc.tile_pool(name="singles", bufs=1))
        zp = es.enter_context(tc.tile_pool(name="zp", bufs=3))
        yp = es.enter_context(tc.tile_pool(name="yp", bufs=4))

        wsb = singles.tile([128, NW * 128], cdt)
        nc.gpsimd.dma_start(wsb, wm[:, :])
        bv = singles.tile([128, NBV], f32)
        nc.gpsimd.dma_start(bv, bvd[:, :])
        xsb = singles.tile([128, 4 * H], cdt)  # whole input resident
        for c in range(4):
            nc.gpsimd.dma_start(xsb[:, c * H:(c + 1) * H],
                                xs[:, c * H:(c + 1) * H])
        h32 = singles.tile([128, H], f32)         # fp32 residual state
        hm0 = singles.tile([128, PAD + H], cdt)   # ping-pong conv mirrors
        hm1 = singles.tile([128, PAD + H], cdt)
        nc.vector.memset(hm0[:, 0:PAD], 0.0)
        nc.vector.memset(hm1[:, 0:PAD], 0.0)
        hm = [hm0, hm1]

        def W(n):
            return wsb[:, n * 128:(n + 1) * 128]

        def bvs(j):
            return bv[:, j:j + 1]

        # ---- input 1x1: h = in_w @ x + in_b (K=256 via 4 chunks of 64)
        with tc.tile_pool(name="psin", bufs=2, space="PSUM") as psin:
            for j in range(NT):
                jb = j * TILE
                cb = PAD + jb
                hp = psin.tile([128, TILE], f32, tag="h0")
                for c in range(4):
                    nc.tensor.matmul(hp, W(c),
                                     xsb[:, c * H + jb: c * H + jb + TILE],
                                     start=(c == 0), stop=(c == 3))
                nc.scalar.activation(h32[:, jb:jb + TILE], hp, AF.Identity,
                                     bias=bvs(0))
                nc.gpsimd.tensor_copy(hm0[:, cb:cb + TILE],
                                      h32[:, jb:jb + TILE])

        # ---- 12 dilated gated residual blocks
        with tc.tile_pool(name="psb", bufs=2, space="PSUM") as psb:
            for i, d in enumerate(DILS):
                wb = 4 + 7 * i
                hin = hm[i % 2]
                hout = hm[(i + 1) % 2]
                for j in range(NT):
                    jb = j * TILE
                    cb = PAD + jb
                    ft = psb.tile([128, TILE], f32, tag="f")
                    nc.tensor.matmul(ft, W(wb + 0),
                                     hin[:, cb - d: cb - d + TILE],
                                     start=True, stop=False)
                    if j == 0:
                        nc.tensor.matmul(ft[:, 0:d], W(wb + 4),
                                         hin[:, PAD + H - d: PAD + H],
                                         start=False, stop=False)
                    nc.tensor.matmul(ft, W(wb + 1), hin[:, cb: cb + TILE],
                                     start=False, stop=True)

                    gt = psb.tile([128, TILE], f32, tag="g")
                    nc.tensor.matmul(gt, W(wb + 2),
                                     hin[:, cb - d: cb - d + TILE],
                                     start=True, stop=False)
                    if j == 0:
                        nc.tensor.matmul(gt[:, 0:d], W(wb + 5),
                                         hin[:, PAD + H - d: PAD + H],
                                         start=False, stop=False)
                    nc.tensor.matmul(gt, W(wb + 3), hin[:, cb: cb + TILE],
                                     start=False, stop=True)

                    tt = zp.tile([128, TILE], cdt, tag="t")
                    nc.scalar.activation(tt, ft, AF.Tanh, bias=bvs(1 + i))
                    st = zp.tile([128, TILE], cdt, tag="s")
                    nc.scalar.activation(st, gt, AF.Sigmoid, bias=bvs(13 + i))
                    zt = zp.tile([128, TILE], cdt, tag="z")
                    nc.vector.tensor_mul(zt, tt, st)

                    ot = psb.tile([128, TILE], f32, tag="ow")
                    nc.tensor.matmul(ot, W(wb + 6), zt, start=True, stop=True)
                    # h32 += ow@z + ob  (in place), then fp16 mirror
                    nc.vector.scalar_tensor_tensor(
                        out=h32[:, jb:jb + TILE], in0=ot,
                        scalar=bvs(25 + i), in1=h32[:, jb:jb + TILE],
                        op0=ADD, op1=ADD)
                    nc.gpsimd.tensor_copy(hout[:, cb:cb + TILE],
                                          h32[:, jb:jb + TILE])

        # ---- output head: relu(o1w@h + o1b), then o2w@r + o2b
        hf = hm[NBLK % 2]
        with tc.tile_pool(name="pso", bufs=4, space="PSUM") as pso:
            for j in range(NT):
                jb = j * TILE
                cb = PAD + jb
                o1t = pso.tile([128, TILE], f32, tag="o1")
                nc.tensor.matmul(o1t, W(88), hf[:, cb:cb + TILE],
                                 start=True, stop=True)
                rt = zp.tile([128, TILE], cdt, tag="r")
                nc.scalar.activation(rt, o1t, AF.Relu, bias=bvs(37))
                for oc in range(2):
                    for half in range(2):
                        o2t = pso.tile([128, TILE], f32, tag="o2")
                        nc.tensor.matmul(o2t, W(89 + 2 * oc + half), rt,
                                         start=True, stop=True)
                        yt = yp.tile([128, TILE], f32, tag="y")
                        if (oc + half) % 2:
                            nc.scalar.activation(yt, o2t, AF.Identity,
                                                 bias=bvs(38 + oc))
                        else:
                            nc.vector.tensor_scalar_add(yt, o2t, bvs(38 + oc))
                        t0 = half * H + jb
                        nc.gpsimd.dma_start(y[oc, :, t0:t0 + TILE], yt)

    nc.finalize()
    _built = nc
    return nc


# ---------------------------------------------------------------- entry points

def _make_in_maps(inputs):
    x = np.asarray(inputs["x"], np.float32)
    inst = np.asarray(inputs["inst"]).astype(np.int64)
    pitch = np.asarray(inputs["pitch"]).astype(np.int64)
    inst_emb = np.asarray(inputs["inst_emb"], np.float32)
    pitch_emb = np.asarray(inputs["pitch_emb"], np.float32)
    cond = np.concatenate([inst_emb[inst], pitch_emb[pitch]], axis=1)  # [B,64]

    wm_host = _pack_weights(
        np.asarray(inputs["in_w"], np.float32),
        np.asarray(inputs["fw"], np.float32),
        np.asarray(inputs["gw"], np.float32),
        np.asarray(inputs["ow"], np.float32),
        np.asarray(inputs["o1w"], np.float32),
        np.asarray(inputs["o2w"], np.float32))

    in_maps = []
    for b in range(B):
        bv_b = _pack_bvec(
            cond[b],
            np.asarray(inputs["in_b"], np.float32),
            np.asarray(inputs["fb"], np.float32),
            np.asarray(inputs["cfw"], np.float32),
            np.asarray(inputs["cfb"], np.float32),
            np.asarray(inputs["gb"], np.float32),
            np.asarray(inputs["cgw"], np.float32),
            np.asarray(inputs["cgb"], np.float32),
            np.asarray(inputs["ob"], np.float32),
            np.asarray(inputs["o1b"], np.float32),
            np.asarray(inputs["o2b"], np.float32))
        in_maps.append({"xs": _pack_x(x[b]), "wm": wm_host, "bv": bv_b})
    return in_maps


def run(inputs, trace=False):
    from concourse.bass_utils import run_bass_kernel_spmd
    nc = _build()
    in_maps = _make_in_maps(inputs)
    res = run_bass_kernel_spmd(nc, in_maps, core_ids=list(range(NCORES)),
                               trace=trace)
    out = np.stack([np.asarray(res.results[b]["y"], np.float32).reshape(NCH, T)
                    for b in range(B)])
    return out, res


def kernel(**inputs) -> np.ndarray:
    out, _ = run(inputs, trace=False)
    return out
